# revision 2
# baseline (speedup 1.0000x reference)
"""Distributed causal-attention-with-dropout kernel for 8 TRN2 NeuronCores.

Strategy (fully static SPMD graph, per-core variance only in input contents):

- Projections are d_out-sharded: core c holds rows [256c, 256c+256) of
  Wq/Wk/Wv, transposed on-chip (cast to bf16 + DMA-transpose) and kept
  SBUF-resident. x is seq-sharded (block c = rows [512c, 512c+512)), cast to
  bf16, AllGathered, and streamed as x^T via transposing DMA. Each core
  produces Q^T/K^T/V^T shards [256, 4096] for its d-slice over ALL seq.
- K^T/V^T shards are AllGathered (d-major concat == natural d order) into
  full K^T/V^T [2048, 4096] bf16, chunked as two column-halves for pipelining.
- Q^T is routed with a single AllToAll so each core ends with Q^T[:, own q].
- Attention is sequence-parallel with causal load balancing: core c owns
  q-tiles {c, 15-c, 16+c, 31-c} (128 rows each). The causal schedule is
  padded to static per-slot k-block counts [2, 4, 6, 8] (20 pairs); causality
  + padding are enforced by per-core thresholds (an input tensor) applied as
  (iota >= thr) * P on the vector engine.
- Softmax without max-subtraction (logits ~ N(0,1), safe in f32): P = exp,
  row-sums accumulated per pair, one reciprocal at the end. Dropout mask is
  multiplied after the causal select; denominators use the pre-dropout sums.
"""

import math
import os
import sys

import numpy as np

for _p in ("/opt/trn_rl_repo", "/root/.axon_site/_ro/trn_rl_repo"):
    if os.path.isdir(_p) and _p not in sys.path:
        sys.path.append(_p)

import concourse.bass as bass
import concourse.tile as tile
from concourse import bacc, mybir
from concourse import bass_utils
from concourse.masks import make_identity

S, D = 4096, 2048
NC = 8
SB = 512          # seq block (projection granularity)
DSH = 256         # d_out shard per core
QT = 128          # q tile rows
KBMAX = (2, 4, 6, 8)
PBASE = (0, 2, 6, 12)
NPAIR = 20
SCALE = 1.0 / math.sqrt(float(D))
F32 = mybir.dt.float32
BF16 = mybir.dt.bfloat16
RG = [list(range(NC))]
ALU = mybir.AluOpType
AFT = mybir.ActivationFunctionType


def owned_tiles(c):
    return (c, 15 - c, 16 + c, 31 - c)


def dest_of_chunk(g):
    """global 128-row q-tile index -> (dest rank, slot)"""
    if g <= 7:
        return g, 0
    if g <= 15:
        return 15 - g, 1
    if g <= 23:
        return g - 16, 2
    return 31 - g, 3


def build():
    nc = bacc.Bacc("TRN2", target_bir_lowering=False, debug=False, num_devices=NC)

    x_in = nc.dram_tensor("x", [SB, D], F32, kind="ExternalInput").ap()
    w_in = {
        w: nc.dram_tensor(w, [DSH, D], F32, kind="ExternalInput").ap()
        for w in ("Wq", "Wk", "Wv")
    }
    mask_in = nc.dram_tensor("drop_mask", [4 * QT, S], F32, kind="ExternalInput").ap()
    sched_in = nc.dram_tensor("sched", [128, NPAIR], F32, kind="ExternalInput").ap()
    out_ext = nc.dram_tensor("out", [4 * QT, D], F32, kind="ExternalOutput").ap()

    with tile.TileContext(nc) as tc:
        # ---------------- DRAM scratch ----------------
        with tc.tile_pool(name="dram", bufs=1, space="DRAM") as dram:
            xb16_d = dram.tile([SB, D], BF16, name="xb16_d")
            wb16_d = {
                w: dram.tile([DSH, D], BF16, name=f"wb16_{w}") for w in w_in
            }
            kt_h = [dram.tile([DSH, 4 * SB], BF16, name=f"kt_h{h}") for h in range(2)]
            vt_h = [dram.tile([DSH, 4 * SB], BF16, name=f"vt_h{h}") for h in range(2)]
            qt_in_d = dram.tile([D, SB], BF16, name="qt_in_d")
            xg_d = dram.tile([S, D], BF16, addr_space="Shared", name="xg_d")
            ktg = [
                dram.tile([D, 4 * SB], BF16, addr_space="Shared", name=f"ktg{h}")
                for h in range(2)
            ]
            vtg = [
                dram.tile([D, 4 * SB], BF16, addr_space="Shared", name=f"vtg{h}")
                for h in range(2)
            ]
            # NB: AllToAll does not support Shared outputs — keep Local.
            qt_out_d = dram.tile([D, SB], BF16, name="qt_out_d")

            # ---------------- constants ----------------
            with tc.tile_pool(name="const", bufs=1) as const:
                sched_sb = const.tile([128, NPAIR], F32, name="sched_sb")
                nc.sync.dma_start(sched_sb[:], sched_in)
                iota_sb = const.tile([128, 512], F32, name="iota_sb")
                nc.gpsimd.iota(
                    iota_sb[:], pattern=[[-1, 512]], base=0, channel_multiplier=1,
                    allow_small_or_imprecise_dtypes=True,
                )
                ident_sb = const.tile([128, 128], BF16, name="ident_sb")
                make_identity(nc, ident_sb[:])

                # ---------------- phase 0: cast x + W to bf16, AllGather x ------
                with tc.tile_pool(name="ld", bufs=2) as ld, \
                     tc.tile_pool(name="cast", bufs=2) as castp:
                    for i in range(4):
                        t = ld.tile([128, D], F32, tag="ld", name=f"xld{i}")
                        nc.sync.dma_start(t[:], x_in[128 * i:128 * (i + 1), :])
                        tb = castp.tile([128, D], BF16, tag="cast", name=f"xcast{i}")
                        nc.vector.tensor_copy(tb[:], t[:])
                        nc.sync.dma_start(xb16_d[128 * i:128 * (i + 1), :], tb[:])
                    nc.gpsimd.collective_compute(
                        "AllGather", ALU.bypass, replica_groups=RG,
                        ins=[xb16_d.opt()], outs=[xg_d.opt()],
                    )
                    for w in w_in:
                        for i in range(2):
                            t = ld.tile([128, D], F32, tag="ld", name=f"wld_{w}{i}")
                            nc.sync.dma_start(t[:], w_in[w][128 * i:128 * (i + 1), :])
                            tb = castp.tile([128, D], BF16, tag="cast",
                                            name=f"wcast_{w}{i}")
                            nc.vector.tensor_copy(tb[:], t[:])
                            nc.sync.dma_start(
                                wb16_d[w][128 * i:128 * (i + 1), :], tb[:])

                # W^T resident tiles via transposing DMA
                with tc.tile_pool(name="wt", bufs=1) as wtp:
                    wt_sb = {}
                    for w in w_in:
                        wt = wtp.tile([128, 16, DSH], BF16, tag=f"wt_{w}",
                                      name=f"wt_{w}")
                        for i in range(16):
                            nc.sync.dma_start(
                                wt[:, i, :],
                                wb16_d[w][:, 128 * i:128 * (i + 1)],
                                transpose=True,
                            )
                        wt_sb[w] = wt

                    # ---------------- phase 1: projections ----------------
                    with tc.tile_pool(name="xt", bufs=2) as xtp, \
                         tc.tile_pool(name="pev", bufs=4) as pev, \
                         tc.tile_pool(name="ppsum", bufs=8, space="PSUM") as pps:
                        for s in range(NC):
                            xt = xtp.tile([128, 16, SB], BF16, tag="xt",
                                          name=f"xt{s}")
                            for i in range(16):
                                nc.sync.dma_start(
                                    xt[:, i, :],
                                    xg_d[SB * s:SB * (s + 1),
                                         128 * i:128 * (i + 1)],
                                    transpose=True,
                                )
                            for w, kind in (("Wq", "q"), ("Wk", "k"), ("Wv", "v")):
                                for m in range(2):
                                    ps = pps.tile([128, SB], F32, tag="ps",
                                                  name=f"ps{s}_{kind}{m}")
                                    for ki in range(16):
                                        nc.tensor.matmul(
                                            ps[:],
                                            lhsT=wt_sb[w][:, ki,
                                                          128 * m:128 * (m + 1)],
                                            rhs=xt[:, ki, :],
                                            start=(ki == 0), stop=(ki == 15),
                                        )
                                    ev = pev.tile([128, SB], BF16, tag="ev",
                                                  name=f"ev{s}_{kind}{m}")
                                    nc.vector.tensor_copy(ev[:], ps[:])
                                    if kind == "q":
                                        for qr in range(4):
                                            g = 4 * s + qr
                                            j, slot = dest_of_chunk(g)
                                            nc.sync.dma_start(
                                                qt_in_d[DSH * j + 128 * m:
                                                        DSH * j + 128 * (m + 1),
                                                        128 * slot:128 * (slot + 1)],
                                                ev[:, 128 * qr:128 * (qr + 1)],
                                            )
                                    else:
                                        dst = kt_h if kind == "k" else vt_h
                                        nc.sync.dma_start(
                                            dst[s // 4][128 * m:128 * (m + 1),
                                                        SB * (s % 4):SB * (s % 4 + 1)],
                                            ev[:],
                                        )
                            if s == 3:
                                for src, dsts in ((kt_h, ktg), (vt_h, vtg)):
                                    nc.gpsimd.collective_compute(
                                        "AllGather", ALU.bypass, replica_groups=RG,
                                        ins=[src[0].opt()], outs=[dsts[0].opt()],
                                    )
                        for src, dsts in ((kt_h, ktg), (vt_h, vtg)):
                            nc.gpsimd.collective_compute(
                                "AllGather", ALU.bypass, replica_groups=RG,
                                ins=[src[1].opt()], outs=[dsts[1].opt()],
                            )
                        nc.gpsimd.collective_compute(
                            "AllToAll", ALU.bypass, replica_groups=RG,
                            ins=[qt_in_d.opt()], outs=[qt_out_d.opt()],
                        )

                # ---------------- phase 2: attention ----------------
                with tc.tile_pool(name="att", bufs=1) as att, \
                     tc.tile_pool(name="ktl", bufs=2) as ktl, \
                     tc.tile_pool(name="vtl", bufs=8) as vtl, \
                     tc.tile_pool(name="mkl", bufs=4) as mkl, \
                     tc.tile_pool(name="pwork", bufs=3) as pwork, \
                     tc.tile_pool(name="oev", bufs=2) as oev, \
                     tc.tile_pool(name="spsum", bufs=2, space="PSUM") as sps, \
                     tc.tile_pool(name="apsum", bufs=1, space="PSUM") as aps, \
                     tc.tile_pool(name="tpsum", bufs=2, space="PSUM") as tps:

                    qt_sb = att.tile([128, 16, SB], BF16, name="qt_sb")
                    nc.sync.dma_start(
                        qt_sb[:], qt_out_d.rearrange("(t p) q -> p t q", p=128))
                    acc = [att.tile([128, D], F32, name=f"acc{t}") for t in range(4)]
                    partials = att.tile([128, NPAIR], F32, name="partials")

                    for kbi in range(8):
                        h, b = kbi // 4, kbi % 4
                        kt = ktl.tile([128, 16, 512], BF16, tag="kt",
                                      name=f"kt{kbi}")
                        nc.sync.dma_start(
                            kt[:],
                            ktg[h][:, 512 * b:512 * (b + 1)]
                            .rearrange("(t p) c -> p t c", p=128),
                        )
                        vcs = []
                        for j in range(4):
                            vc = vtl.tile([128, D], BF16, tag="vc",
                                          name=f"vc{kbi}_{j}")
                            nc.sync.dma_start(
                                vc[:],
                                vtg[h][:, 512 * b + 128 * j:512 * b + 128 * (j + 1)],
                                transpose=True,
                            )
                            vcs.append(vc)
                        for slot in range(4):
                            if kbi >= KBMAX[slot]:
                                continue
                            p = PBASE[slot] + kbi
                            mk = mkl.tile([128, 512], F32, tag="mk",
                                          name=f"mk{kbi}_{slot}")
                            nc.sync.dma_start(
                                mk[:],
                                mask_in[128 * slot:128 * (slot + 1),
                                        512 * kbi:512 * (kbi + 1)],
                            )
                            sc = sps.tile([128, 512], F32, tag="sc",
                                          name=f"sc{kbi}_{slot}")
                            for ki in range(16):
                                nc.tensor.matmul(
                                    sc[:],
                                    lhsT=qt_sb[:, ki, 128 * slot:128 * (slot + 1)],
                                    rhs=kt[:, ki, :],
                                    start=(ki == 0), stop=(ki == 15),
                                )
                            pex = pwork.tile([128, 512], BF16, tag="pex",
                                             name=f"pex{kbi}_{slot}")
                            nc.scalar.activation(pex[:], sc[:], AFT.Exp, scale=SCALE)
                            pcs = pwork.tile([128, 512], BF16, tag="pcs",
                                             name=f"pcs{kbi}_{slot}")
                            nc.vector.scalar_tensor_tensor(
                                out=pcs[:], in0=iota_sb[:],
                                scalar=sched_sb[:, p:p + 1], in1=pex[:],
                                op0=ALU.is_ge, op1=ALU.mult,
                                accum_out=partials[:, p:p + 1],
                            )
                            pm = pwork.tile([128, 512], BF16, tag="pm",
                                            name=f"pm{kbi}_{slot}")
                            nc.vector.tensor_mul(pm[:], pcs[:], mk[:])
                            pmt = pwork.tile([128, 4, 128], BF16, tag="pmt",
                                             name=f"pmt{kbi}_{slot}")
                            for j in range(4):
                                tp = tps.tile([128, 128], BF16, tag="tp",
                                              name=f"tp{kbi}_{slot}{j}")
                                nc.tensor.transpose(
                                    tp[:], pm[:, 128 * j:128 * (j + 1)], ident_sb[:])
                                nc.vector.tensor_copy(pmt[:, j, :], tp[:])
                            av = aps.tile([128, D], F32, tag="av",
                                          name=f"av{kbi}_{slot}")
                            for j in range(4):
                                for n in range(4):
                                    nc.tensor.matmul(
                                        av[:, 512 * n:512 * (n + 1)],
                                        lhsT=pmt[:, j, :],
                                        rhs=vcs[j][:, 512 * n:512 * (n + 1)],
                                        start=(j == 0), stop=(j == 3),
                                        skip_group_check=True,
                                    )
                            if kbi == 0:
                                nc.vector.tensor_copy(acc[slot][:], av[:])
                            else:
                                nc.vector.scalar_tensor_tensor(
                                    out=acc[slot][:], in0=av[:], scalar=1.0,
                                    in1=acc[slot][:], op0=ALU.mult, op1=ALU.add,
                                )

                    # normalize + write out
                    den = att.tile([128, 4], F32, name="den")
                    rec = att.tile([128, 4], F32, name="rec")
                    for slot in range(4):
                        nc.vector.tensor_reduce(
                            den[:, slot:slot + 1],
                            partials[:, PBASE[slot]:PBASE[slot] + KBMAX[slot]],
                            axis=mybir.AxisListType.X, op=ALU.add,
                        )
                        nc.vector.reciprocal(rec[:, slot:slot + 1],
                                             den[:, slot:slot + 1])
                        osb = oev.tile([128, D], F32, tag="osb", name=f"osb{slot}")
                        nc.vector.tensor_scalar_mul(
                            osb[:], acc[slot][:], rec[:, slot:slot + 1])
                        nc.sync.dma_start(
                            out_ext[128 * slot:128 * (slot + 1), :], osb[:])

    nc.compile()
    return nc


_NC_CACHE = None


def _get_nc():
    global _NC_CACHE
    if _NC_CACHE is None:
        _NC_CACHE = build()
    return _NC_CACHE


def make_in_maps(x, Wq, Wk, Wv, drop_mask):
    x = np.ascontiguousarray(np.asarray(x, dtype=np.float32))
    Wq = np.ascontiguousarray(np.asarray(Wq, dtype=np.float32))
    Wk = np.ascontiguousarray(np.asarray(Wk, dtype=np.float32))
    Wv = np.ascontiguousarray(np.asarray(Wv, dtype=np.float32))
    drop_mask = np.ascontiguousarray(np.asarray(drop_mask, dtype=np.float32))
    in_maps = []
    for c in range(NC):
        tl = owned_tiles(c)
        thr = np.array(
            [
                (512 * kbi - 128 * tl[slot])
                if kbi < (tl[slot] // 4 + 1) else 1.0e9
                for slot in range(4) for kbi in range(KBMAX[slot])
            ],
            dtype=np.float32,
        )
        in_maps.append({
            "x": x[SB * c:SB * (c + 1)],
            "Wq": Wq[DSH * c:DSH * (c + 1)],
            "Wk": Wk[DSH * c:DSH * (c + 1)],
            "Wv": Wv[DSH * c:DSH * (c + 1)],
            "drop_mask": np.ascontiguousarray(
                np.concatenate(
                    [drop_mask[128 * t:128 * (t + 1)] for t in tl], axis=0)),
            "sched": np.ascontiguousarray(np.tile(thr[None, :], (128, 1))),
        })
    return in_maps


def assemble(results):
    full = np.zeros((S, D), dtype=np.float32)
    for c in range(NC):
        o = results[c]["out"]
        for slot, t in enumerate(owned_tiles(c)):
            full[128 * t:128 * (t + 1)] = o[128 * slot:128 * (slot + 1)]
    return full


def kernel(x, Wq, Wk, Wv, drop_mask):
    nc = _get_nc()
    in_maps = make_in_maps(x, Wq, Wk, Wv, drop_mask)
    res = bass_utils.run_bass_kernel_spmd(nc, in_maps, core_ids=list(range(NC)))
    return assemble(res.results)


def kernel_profiled(x, Wq, Wk, Wv, drop_mask):
    """Like kernel(), but captures an NTFF profile; returns (out, exec_time_ns,
    trace_path)."""
    nc = _get_nc()
    in_maps = make_in_maps(x, Wq, Wk, Wv, drop_mask)
    res = bass_utils.run_bass_kernel_spmd(
        nc, in_maps, core_ids=list(range(NC)), trace=True)
    trace_path = None
    if res.instructions_and_trace is not None:
        trace_path = res.instructions_and_trace[1]
    return assemble(res.results), res.exec_time_ns, trace_path


# revision 4
# speedup vs baseline: 1.2883x; 1.2883x over previous
"""Distributed causal-attention-with-dropout kernel for 8 TRN2 NeuronCores.

Strategy (fully static SPMD graph, per-core variance only in input contents):

- Projections are d_out-sharded: core c holds rows [256c, 256c+256) of
  Wq/Wk/Wv, transposed on-chip (cast to bf16 + one whole-tensor DMA-transpose)
  and kept SBUF-resident. x is seq-sharded (block c = rows [512c, 512c+512)),
  cast to bf16, AllGathered in two column halves, and streamed as x^T via
  whole-block transposing DMAs. Each core produces Q^T/K^T/V^T shards
  [256, 4096] for its d-slice over ALL seq.
- K^T/V^T shards are AllGathered (d-major concat == natural d order) into
  full K^T/V^T [2048, 4096] bf16, chunked as two column-halves for pipelining.
- Q^T is routed with a single AllToAll so each core ends with Q^T[:, own q].
- Attention is sequence-parallel with causal load balancing: core c owns
  q-tiles {c, 15-c, 16+c, 31-c} (128 rows each). The causal schedule is
  padded to static per-slot k-block counts [2, 4, 6, 8] (20 pairs); causality
  + padding are enforced by per-core thresholds (an input tensor) applied as
  (iota >= thr) * P on the vector engine.
- Softmax without max-subtraction (logits ~ N(0,1), safe in f32): P = exp,
  row-sums accumulated per pair, one reciprocal at the end. Dropout mask is
  multiplied after the causal select; denominators use the pre-dropout sums.
"""

import math
import os
import sys

import numpy as np

for _p in ("/opt/trn_rl_repo", "/root/.axon_site/_ro/trn_rl_repo"):
    if os.path.isdir(_p) and _p not in sys.path:
        sys.path.append(_p)

import concourse.bass as bass
import concourse.tile as tile
from concourse import bacc, mybir
from concourse import bass_utils
from concourse.masks import make_identity

S, D = 4096, 2048
NC = 8
SB = 512          # seq block (projection granularity)
DSH = 256         # d_out shard per core
HD = D // 2       # x all-gather column half
KBMAX = (2, 4, 6, 8)
PBASE = (0, 2, 6, 12)
# first active slot per k-block index (KBMAX is ascending)
SLOT0 = [0, 0, 1, 1, 2, 2, 3, 3]
NPAIR = 20
SCALE = 1.0 / math.sqrt(float(D))
F32 = mybir.dt.float32
BF16 = mybir.dt.bfloat16
RG = [list(range(NC))]
ALU = mybir.AluOpType
AFT = mybir.ActivationFunctionType


def owned_tiles(c):
    return (c, 15 - c, 16 + c, 31 - c)


def zone_info(s):
    """For projection seq-block s (chunks 4s..4s+3), the Q^T eviction routing:
    returns (slot, j0, jstep): quarter q goes to dest rank j0 + q*jstep,
    column slot*128."""
    g0 = 4 * s
    if g0 <= 7:
        return 0, g0, 1
    if g0 <= 15:
        return 1, 15 - g0, -1
    if g0 <= 23:
        return 2, g0 - 16, 1
    return 3, 31 - g0, -1


def build():
    nc = bacc.Bacc("TRN2", target_bir_lowering=False, debug=False, num_devices=NC)

    x_in = nc.dram_tensor("x", [SB, D], F32, kind="ExternalInput").ap()
    w_in = {
        w: nc.dram_tensor(w, [DSH, D], F32, kind="ExternalInput").ap()
        for w in ("Wq", "Wk", "Wv")
    }
    mask_in = nc.dram_tensor("drop_mask", [4 * 128, S], F32,
                             kind="ExternalInput").ap()
    sched_in = nc.dram_tensor("sched", [128, NPAIR], F32, kind="ExternalInput").ap()
    out_ext = nc.dram_tensor("out", [4 * 128, D], F32, kind="ExternalOutput").ap()

    with tile.TileContext(nc) as tc:
        # ---------------- DRAM scratch ----------------
        with tc.tile_pool(name="dram", bufs=1, space="DRAM") as dram:
            xb16 = [dram.tile([SB, HD], BF16, name=f"xb16_{h}") for h in range(2)]
            wb16_d = {w: dram.tile([DSH, D], BF16, name=f"wb16_{w}") for w in w_in}
            kt_h = [dram.tile([DSH, 4 * SB], BF16, name=f"kt_h{h}") for h in range(2)]
            vt_h = [dram.tile([DSH, 4 * SB], BF16, name=f"vt_h{h}") for h in range(2)]
            qt_in_d = dram.tile([D, SB], BF16, name="qt_in_d")
            xg = [
                dram.tile([S, HD], BF16, addr_space="Shared", name=f"xg_{h}")
                for h in range(2)
            ]
            ktg = [
                dram.tile([D, 4 * SB], BF16, addr_space="Shared", name=f"ktg{h}")
                for h in range(2)
            ]
            vtg = [
                dram.tile([D, 4 * SB], BF16, addr_space="Shared", name=f"vtg{h}")
                for h in range(2)
            ]
            # NB: AllToAll does not support Shared outputs — keep Local.
            qt_out_d = dram.tile([D, SB], BF16, name="qt_out_d")

            # ---------------- constants ----------------
            with tc.tile_pool(name="const", bufs=1) as const:
                sched_sb = const.tile([128, NPAIR], F32, name="sched_sb")
                nc.scalar.dma_start(sched_sb[:], sched_in)
                iota_sb = const.tile([128, 512], F32, name="iota_sb")
                nc.gpsimd.iota(
                    iota_sb[:], pattern=[[-1, 512]], base=0, channel_multiplier=1,
                    allow_small_or_imprecise_dtypes=True,
                )
                ident_sb = const.tile([128, 128], BF16, name="ident_sb")
                make_identity(nc, ident_sb[:])

                # ---------------- phase 0: cast x + W to bf16, AllGather x ------
                with tc.tile_pool(name="ld", bufs=2) as ld, \
                     tc.tile_pool(name="cast", bufs=2) as castp:
                    # x: one load [128, 4, 2048], one cast, two half writes
                    xt_f32 = ld.tile([128, 4, D], F32, tag="ld", name="x_f32")
                    nc.scalar.dma_start(
                        xt_f32[:], x_in.rearrange("(t p) d -> p t d", p=128))
                    xt_b16 = castp.tile([128, 4, D], BF16, tag="cast", name="x_b16")
                    nc.vector.tensor_copy(xt_b16[:], xt_f32[:])
                    for h in range(2):
                        nc.scalar.dma_start(
                            xb16[h].rearrange("(t p) d -> p t d", p=128),
                            xt_b16[:, :, HD * h:HD * (h + 1)],
                        )
                    for h in range(2):
                        nc.gpsimd.collective_compute(
                            "AllGather", ALU.bypass, replica_groups=RG,
                            ins=[xb16[h].opt()], outs=[xg[h].opt()],
                        )
                    for w in w_in:
                        wf = ld.tile([128, 2, D], F32, tag="ld", name=f"wf_{w}")
                        nc.scalar.dma_start(
                            wf[:], w_in[w].rearrange("(t p) d -> p t d", p=128))
                        wb = castp.tile([128, 2, D], BF16, tag="cast",
                                        name=f"wb_{w}")
                        nc.vector.tensor_copy(wb[:], wf[:])
                        nc.scalar.dma_start(
                            wb16_d[w].rearrange("(t p) d -> p t d", p=128), wb[:])

                # W^T resident tiles: one whole-tensor transposing DMA each
                with tc.tile_pool(name="wt", bufs=1) as wtp:
                    wt_sb = {}
                    for w in w_in:
                        wt = wtp.tile([128, 16, DSH], BF16, tag=f"wt_{w}",
                                      name=f"wt_{w}")
                        nc.sync.dma_start(wt[:], wb16_d[w][:], transpose=True)
                        wt_sb[w] = wt

                    # ---------------- phase 1: projections ----------------
                    with tc.tile_pool(name="xt", bufs=2) as xtp, \
                         tc.tile_pool(name="pev", bufs=4) as pev, \
                         tc.tile_pool(name="ppsum", bufs=8, space="PSUM") as pps:
                        for s in range(NC):
                            xt = xtp.tile([128, 16, SB], BF16, tag="xt",
                                          name=f"xt{s}")
                            for h in range(2):
                                nc.sync.dma_start(
                                    xt[:, 8 * h:8 * (h + 1), :],
                                    xg[h][SB * s:SB * (s + 1), :],
                                    transpose=True,
                                )
                            for w, kind in (("Wq", "q"), ("Wk", "k"), ("Wv", "v")):
                                for m in range(2):
                                    ps = pps.tile([128, SB], F32, tag="ps",
                                                  name=f"ps{s}_{kind}{m}")
                                    for ki in range(16):
                                        nc.tensor.matmul(
                                            ps[:],
                                            lhsT=wt_sb[w][:, ki,
                                                          128 * m:128 * (m + 1)],
                                            rhs=xt[:, ki, :],
                                            start=(ki == 0), stop=(ki == 15),
                                        )
                                    ev = pev.tile([128, SB], BF16, tag="ev",
                                                  name=f"ev{s}_{kind}{m}")
                                    nc.vector.tensor_copy(ev[:], ps[:])
                                    if kind == "q":
                                        slot, j0, jstep = zone_info(s)
                                        # one DMA: quarter q -> rank j0+q*jstep
                                        dst = qt_in_d.rearrange(
                                            "(j r) c -> j r c", r=DSH)[j0::jstep]
                                        nc.scalar.dma_start(
                                            dst[0:4,
                                                128 * m:128 * (m + 1),
                                                128 * slot:128 * (slot + 1)]
                                            .rearrange("q p c -> p q c"),
                                            ev[:].rearrange(
                                                "p (q c) -> p q c", q=4),
                                        )
                                    else:
                                        dstt = kt_h if kind == "k" else vt_h
                                        nc.scalar.dma_start(
                                            dstt[s // 4][128 * m:128 * (m + 1),
                                                         SB * (s % 4):
                                                         SB * (s % 4 + 1)],
                                            ev[:],
                                        )
                            if s == 3:
                                for src, dsts in ((kt_h, ktg), (vt_h, vtg)):
                                    nc.gpsimd.collective_compute(
                                        "AllGather", ALU.bypass, replica_groups=RG,
                                        ins=[src[0].opt()], outs=[dsts[0].opt()],
                                    )
                        # A2A first: attention needs Q^T before late k-blocks
                        nc.gpsimd.collective_compute(
                            "AllToAll", ALU.bypass, replica_groups=RG,
                            ins=[qt_in_d.opt()], outs=[qt_out_d.opt()],
                        )
                        for src, dsts in ((kt_h, ktg), (vt_h, vtg)):
                            nc.gpsimd.collective_compute(
                                "AllGather", ALU.bypass, replica_groups=RG,
                                ins=[src[1].opt()], outs=[dsts[1].opt()],
                            )

                # ---------------- phase 2: attention ----------------
                with tc.tile_pool(name="att", bufs=1) as att, \
                     tc.tile_pool(name="ktl", bufs=2) as ktl, \
                     tc.tile_pool(name="vtl", bufs=2) as vtl, \
                     tc.tile_pool(name="mkl", bufs=2) as mkl, \
                     tc.tile_pool(name="pwork", bufs=3) as pwork, \
                     tc.tile_pool(name="oev", bufs=2) as oev, \
                     tc.tile_pool(name="spsum", bufs=2, space="PSUM") as sps, \
                     tc.tile_pool(name="apsum", bufs=1, space="PSUM") as aps, \
                     tc.tile_pool(name="tpsum", bufs=2, space="PSUM") as tps:

                    qt_sb = att.tile([128, 16, SB], BF16, name="qt_sb")
                    nc.scalar.dma_start(
                        qt_sb[:], qt_out_d.rearrange("(t p) q -> p t q", p=128))
                    acc = [att.tile([128, D], F32, name=f"acc{t}") for t in range(4)]
                    partials = att.tile([128, NPAIR], F32, name="partials")

                    for kbi in range(8):
                        h, b = kbi // 4, kbi % 4
                        kt = ktl.tile([128, 16, 512], BF16, tag="kt",
                                      name=f"kt{kbi}")
                        nc.scalar.dma_start(
                            kt[:],
                            ktg[h][:, 512 * b:512 * (b + 1)]
                            .rearrange("(t p) c -> p t c", p=128),
                        )
                        # one whole-block transposing DMA: V chunks [128,4,2048]
                        vt4 = vtl.tile([128, 4, D], BF16, tag="vc",
                                       name=f"vt4_{kbi}")
                        nc.sync.dma_start(
                            vt4[:], vtg[h][:, 512 * b:512 * (b + 1)],
                            transpose=True,
                        )
                        # one mask DMA for all active slots of this k-block
                        s0 = SLOT0[kbi]
                        nact = 4 - s0
                        mk = mkl.tile([128, 4, 512], F32, tag="mk",
                                      name=f"mk{kbi}")
                        nc.scalar.dma_start(
                            mk[:, s0:4, :],
                            mask_in[128 * s0:512, 512 * kbi:512 * (kbi + 1)]
                            .rearrange("(t p) c -> p t c", p=128),
                        )
                        for slot in range(s0, 4):
                            p = PBASE[slot] + kbi
                            sc = sps.tile([128, 512], F32, tag="sc",
                                          name=f"sc{kbi}_{slot}")
                            for ki in range(16):
                                nc.tensor.matmul(
                                    sc[:],
                                    lhsT=qt_sb[:, ki, 128 * slot:128 * (slot + 1)],
                                    rhs=kt[:, ki, :],
                                    start=(ki == 0), stop=(ki == 15),
                                )
                            pex = pwork.tile([128, 512], BF16, tag="pex",
                                             name=f"pex{kbi}_{slot}")
                            nc.scalar.activation(pex[:], sc[:], AFT.Exp, scale=SCALE)
                            pcs = pwork.tile([128, 512], BF16, tag="pcs",
                                             name=f"pcs{kbi}_{slot}")
                            nc.vector.scalar_tensor_tensor(
                                out=pcs[:], in0=iota_sb[:],
                                scalar=sched_sb[:, p:p + 1], in1=pex[:],
                                op0=ALU.is_ge, op1=ALU.mult,
                                accum_out=partials[:, p:p + 1],
                            )
                            pm = pwork.tile([128, 512], BF16, tag="pm",
                                            name=f"pm{kbi}_{slot}")
                            nc.vector.tensor_mul(pm[:], pcs[:], mk[:, slot, :])
                            pmt = pwork.tile([128, 4, 128], BF16, tag="pmt",
                                             name=f"pmt{kbi}_{slot}")
                            for j in range(4):
                                tp = tps.tile([128, 128], BF16, tag="tp",
                                              name=f"tp{kbi}_{slot}{j}")
                                nc.tensor.transpose(
                                    tp[:], pm[:, 128 * j:128 * (j + 1)], ident_sb[:])
                                nc.vector.tensor_copy(pmt[:, j, :], tp[:])
                            av = aps.tile([128, D], F32, tag="av",
                                          name=f"av{kbi}_{slot}")
                            for j in range(4):
                                for n in range(4):
                                    nc.tensor.matmul(
                                        av[:, 512 * n:512 * (n + 1)],
                                        lhsT=pmt[:, j, :],
                                        rhs=vt4[:, j, 512 * n:512 * (n + 1)],
                                        start=(j == 0), stop=(j == 3),
                                        skip_group_check=True,
                                    )
                            if kbi == 0:
                                nc.vector.tensor_copy(acc[slot][:], av[:])
                            else:
                                nc.vector.scalar_tensor_tensor(
                                    out=acc[slot][:], in0=av[:], scalar=1.0,
                                    in1=acc[slot][:], op0=ALU.mult, op1=ALU.add,
                                )

                    # normalize + write out
                    den = att.tile([128, 4], F32, name="den")
                    rec = att.tile([128, 4], F32, name="rec")
                    for slot in range(4):
                        nc.vector.tensor_reduce(
                            den[:, slot:slot + 1],
                            partials[:, PBASE[slot]:PBASE[slot] + KBMAX[slot]],
                            axis=mybir.AxisListType.X, op=ALU.add,
                        )
                        nc.vector.reciprocal(rec[:, slot:slot + 1],
                                             den[:, slot:slot + 1])
                        osb = oev.tile([128, D], F32, tag="osb", name=f"osb{slot}")
                        nc.vector.tensor_scalar_mul(
                            osb[:], acc[slot][:], rec[:, slot:slot + 1])
                        nc.scalar.dma_start(
                            out_ext[128 * slot:128 * (slot + 1), :], osb[:])

    nc.compile()
    return nc


_NC_CACHE = None


def _get_nc():
    global _NC_CACHE
    if _NC_CACHE is None:
        _NC_CACHE = build()
    return _NC_CACHE


def make_in_maps(x, Wq, Wk, Wv, drop_mask):
    x = np.ascontiguousarray(np.asarray(x, dtype=np.float32))
    Wq = np.ascontiguousarray(np.asarray(Wq, dtype=np.float32))
    Wk = np.ascontiguousarray(np.asarray(Wk, dtype=np.float32))
    Wv = np.ascontiguousarray(np.asarray(Wv, dtype=np.float32))
    drop_mask = np.ascontiguousarray(np.asarray(drop_mask, dtype=np.float32))
    in_maps = []
    for c in range(NC):
        tl = owned_tiles(c)
        thr = np.array(
            [
                (512 * kbi - 128 * tl[slot])
                if kbi < (tl[slot] // 4 + 1) else 1.0e9
                for slot in range(4) for kbi in range(KBMAX[slot])
            ],
            dtype=np.float32,
        )
        in_maps.append({
            "x": x[SB * c:SB * (c + 1)],
            "Wq": Wq[DSH * c:DSH * (c + 1)],
            "Wk": Wk[DSH * c:DSH * (c + 1)],
            "Wv": Wv[DSH * c:DSH * (c + 1)],
            "drop_mask": np.ascontiguousarray(
                np.concatenate(
                    [drop_mask[128 * t:128 * (t + 1)] for t in tl], axis=0)),
            "sched": np.ascontiguousarray(np.tile(thr[None, :], (128, 1))),
        })
    return in_maps


def assemble(results):
    full = np.zeros((S, D), dtype=np.float32)
    for c in range(NC):
        o = results[c]["out"]
        for slot, t in enumerate(owned_tiles(c)):
            full[128 * t:128 * (t + 1)] = o[128 * slot:128 * (slot + 1)]
    return full


def kernel(x, Wq, Wk, Wv, drop_mask):
    nc = _get_nc()
    in_maps = make_in_maps(x, Wq, Wk, Wv, drop_mask)
    res = bass_utils.run_bass_kernel_spmd(nc, in_maps, core_ids=list(range(NC)))
    return assemble(res.results)


def kernel_profiled(x, Wq, Wk, Wv, drop_mask):
    """Like kernel(), but captures an NTFF profile; returns (out, exec_time_ns,
    trace_path)."""
    nc = _get_nc()
    in_maps = make_in_maps(x, Wq, Wk, Wv, drop_mask)
    res = bass_utils.run_bass_kernel_spmd(
        nc, in_maps, core_ids=list(range(NC)), trace=True)
    trace_path = None
    if res.instructions_and_trace is not None:
        trace_path = res.instructions_and_trace[1]
    return assemble(res.results), res.exec_time_ns, trace_path
